# revision 34
# baseline (speedup 1.0000x reference)
"""Trainium2 Bass kernel for pairwise-MLP GNN message passing.

Computation (per batch b, position l):
    x[i,j] = concat(states[l,i], states[l,j])           # [N,N,2D]
    out    = sigmoid(MLP(x))                            # [N,N,8], MLP: 32->64->64->8

Factorization used on device: the first linear layer splits into
A = states @ W1[:D] + b1 and B = states @ W1[D:], so
h1[i,j] = relu(A[i] + B[j]) — the N^2 expansion happens as a cheap
broadcast add on the vector engine instead of an N^2-row matmul.

Sharding: data-parallel over batch, core c <- batch c (8 cores, B=8).

Device layout (per core, L=64 l-blocks, 2 l-blocks = 1 "sb" superblock,
2 sbs = 1 "pair"):
  - features live on partitions: partitions 0:64 = even l-block of the sb,
    64:128 = odd l-block (via a device-side shifted DMA of states^T feeding
    block-diagonal-packed matmuls).
  - pair columns col = 32*i + j, 1024 per l-block.
  - L2/L3 run as concurrent 64x64 / 64x32 tile_position matmuls.
  - h2 eviction: column-half a always on ScalarE; column-half b alternates
    ScalarE (even sb) / VectorE (odd sb) so the engines balance while each
    L3 chunk still depends on exactly one evicting engine.
  - Output leaves the device as bf16 sigmoid(z) in [16 pairs, 4 groups,
    8 f, 1024] per core; the host upcasts and inverts the layout while
    unsharding.
"""

import os
import sys

import numpy as np

for _p in ("/opt/trn_rl_repo", "/root/.axon_site/_ro/trn_rl_repo"):
    if os.path.isdir(_p) and _p not in sys.path:
        sys.path.insert(0, _p)

from concourse import bacc, mybir, tile
from concourse.bass_utils import run_bass_kernel_spmd

B, L, N, D = 8, 64, 32, 16
H = 64            # hidden width (h1 and h2)
F = 8             # out_dim
NCORES = 8
NSB = L // 2      # 32 superblocks per core
NPAIR = NSB // 2  # 16 pairs per core
COLS = N * N      # 1024 pair columns per l-block

FP32 = mybir.dt.float32
BF16 = mybir.dt.bfloat16
NP_BF16 = mybir.dt.np(BF16)

_PROGRAM = None  # (nc, input_names)
LAST_RESULT = None  # BassKernelResults of the most recent kernel() call


def _build_program():
    nc = bacc.Bacc("TRN2", target_bir_lowering=False, debug=False)

    d_states = nc.dram_tensor("statesT", [16, 2048], BF16, kind="ExternalInput").ap()
    d_Wpack = nc.dram_tensor("Wpack", [128, 224], BF16, kind="ExternalInput").ap()
    d_biases = nc.dram_tensor("biases", [128, 3], FP32, kind="ExternalInput").ap()
    d_out = nc.dram_tensor(
        "out", [NPAIR // 2, 4, F, 2, COLS], BF16, kind="ExternalOutput"
    ).ap()

    add = mybir.AluOpType.add
    max_ = mybir.AluOpType.max
    AF = mybir.ActivationFunctionType

    with tile.TileContext(nc) as tc:
        with tc.tile_pool(name="const", bufs=1) as const_pool:
            statesQ = const_pool.tile([48, 2048], BF16, name="statesQ_t")[:]
            Wpack = const_pool.tile([128, 224], BF16, name="Wpack_t")[:]
            biases = const_pool.tile([128, 3], FP32, name="biases_t")[:]
            Wl1 = Wpack[0:48, 0:128]
            W2q = Wpack[:, 128:192]
            W3q = Wpack[:, 192:224]
            bias1 = biases[:, 0:1]
            bias2 = biases[:, 1:2]
            bias3 = biases[:, 2:3]
            A2dup = const_pool.tile([128, 2 * COLS], BF16, name="A2dup_t")[:]
            B2s = const_pool.tile([128, COLS], BF16, name="B2s_t")[:]
            junk = const_pool.tile([128, 512], BF16, name="junk_t")[:]

            # statesQ rows 0:16 = statesT, rows 32:48 = statesT shifted left by
            # one l-slot (32 cols) — built from the same DRAM region.
            # statesQ split into halves so the first L1 matmuls (which only
            # need cols 0:1024) start before the full load completes.
            nc.sync.dma_start(out=statesQ[0:16, 0:1024], in_=d_states[:, 0:1024])
            nc.sync.dma_start(out=statesQ[32:48, 0:1024], in_=d_states[:, 32:1056])
            nc.sync.dma_start(out=Wpack, in_=d_Wpack)
            nc.sync.dma_start(out=biases, in_=d_biases)
            nc.sync.dma_start(out=statesQ[0:16, 1024:2048], in_=d_states[:, 1024:2048])
            nc.sync.dma_start(out=statesQ[32:48, 1024:2016], in_=d_states[:, 1056:2048])

            # ---- Layer 1: A2/B2 = per-agent halves of the first linear layer.
            # A2[p, 32*sb + i]: p<64 -> even l-block (2sb), p>=64 -> odd (2sb+1)
            # via the shifted rows 32:48 of statesQ.
            with tc.tile_pool(name="abps", bufs=1, space="PSUM") as ab_pool:
                A2ps = ab_pool.tile([128, COLS], FP32, tag="a2", name="A2ps_t")[:]
                B2ps = ab_pool.tile([128, COLS], FP32, tag="b2", name="B2ps_t")[:]
                warm = ab_pool.tile([128, 512], FP32, tag="warm", name="warm_t")[:]
                rhs_even = statesQ[0:16].rearrange("p (s c) -> p s c", s=32)
                rhs_odd = statesQ[32:48].rearrange("p (s c) -> p s c", s=32)
                # HAM warmup: ~4.7us of back-to-back FULL-ARRAY matmuls
                # (128x128 x N=512, junk data with no DMA dependency) flips
                # the PE clock gate to 8/8 (2.4 GHz) during the input-DMA
                # window, before the real matmuls start; once warm, the
                # steady state's short gaps keep it warm. Results unread.
                nc.gpsimd.memset(junk, 0.5)
                for _ in range(11):
                    nc.tensor.matmul(warm, junk[:, 0:128], junk)
                # sbh outer so the first four matmuls only need the first
                # statesQ half (cols 0:1024 = superblocks 0:16).
                for sbh in (0, 1):
                    for w_lo, ps in ((0, A2ps), (64, B2ps)):
                        for half, rhs in ((0, rhs_even), (1, rhs_odd)):
                            lhsT = Wl1[32 * half : 32 * half + 16, w_lo : w_lo + 64]
                            nc.tensor.matmul(
                                ps[64 * half : 64 * half + 64, 512 * sbh : 512 * sbh + 512],
                                lhsT,
                                rhs[:, 16 * sbh : 16 * sbh + 16, 0:32],
                            )
                # Bridge warmups: keep the PE busy through the pipeline-fill
                # window (A2/B2 eviction + first broadcast add) so the HAM
                # never sees a full idle window and re-throttles.
                for _ in range(14):
                    nc.tensor.matmul(warm, junk[:, 0:128], junk)
                # Evict A2 twice (duplicated pairs so the later broadcast add
                # keeps an innermost unit stride), folding in b1. The first 256
                # cols (pairs 0-3) go as small early ops (ScalarE/VectorE in
                # parallel) so the first broadcast add starts ASAP; the
                # remainders all run on ScalarE, keeping VectorE's queue free
                # for the first expansion adds.
                dupview = A2dup.rearrange("p (c two) -> p two c", two=2)
                nc.scalar.activation(dupview[:, 0, 0:256], A2ps[:, 0:256], AF.Identity, bias=bias1)
                nc.vector.tensor_scalar(dupview[:, 1, 0:256], A2ps[:, 0:256], bias1, 0.0, add, add)
                nc.vector.tensor_copy(B2s[:, 0:256], B2ps[:, 0:256])
                nc.scalar.activation(dupview[:, 0, 256:1024], A2ps[:, 256:1024], AF.Identity, bias=bias1)
                nc.scalar.activation(dupview[:, 1, 256:1024], A2ps[:, 256:1024], AF.Identity, bias=bias1)
                nc.scalar.activation(B2s[:, 256:1024], B2ps[:, 256:1024], AF.Identity)

            with (
                tc.tile_pool(name="work", bufs=6) as work_pool,
                tc.tile_pool(name="sigp", bufs=2) as sig_pool,
                tc.tile_pool(name="l2ps", bufs=2, space="PSUM") as l2_pool,
                tc.tile_pool(name="l3ps", bufs=2, space="PSUM") as l3_pool,
            ):
                # Software pipeline, 1 sb deep: PE order is
                # L2(0), L3(0), L2(1), L3(1), ... so L3(sb) (which needs
                # S2(sb)'s eviction) never blocks L2(sb+1) in the strict-FIFO
                # PE queue.
                h2_tiles = {}     # sb -> h2 AP
                psum3_tiles = {}  # pair -> psum3 AP
                sig_tiles = {}    # pairgroup -> sig AP
                filler_tiles = {}  # pair -> evicted psum3 AP (for keep-warm)

                def emit_s1(pair):
                    h1pre = work_pool.tile([128, 2 * COLS], BF16, tag="h1pre", name="h1pre_t")[:]
                    h1 = work_pool.tile([128, 2 * COLS], BF16, tag="h1", name="h1_t")[:]
                    a_in = (
                        A2dup[:, 128 * pair : 128 * pair + 128]
                        .rearrange("p (s i two) -> p s i two", s=2, two=2)
                        .unsqueeze(3)
                        .broadcast_to([128, 2, 32, 16, 2])
                    )
                    b_in = (
                        B2s[:, 64 * pair : 64 * pair + 64]
                        .rearrange("p (s jh jl) -> p s jh jl", s=2, jl=2)
                        .unsqueeze(2)
                        .broadcast_to([128, 2, 32, 16, 2])
                    )
                    h1pre_v = h1pre.rearrange(
                        "p (s i jh jl) -> p s i jh jl", s=2, i=32, jl=2
                    )
                    nc.vector.tensor_add(h1pre_v, a_in, b_in)
                    nc.vector.tensor_scalar_max(h1, h1pre, 0.0)
                    return h1

                def emit_l2_s2(sb, hk):
                    # Column-half a: evicted by ScalarE; column-half b
                    # alternates ScalarE (even sb) / VectorE (odd sb).
                    # Separate PSUM banks + separate h2 tiles keep each L3
                    # chunk dependent on exactly one evicting engine.
                    psum2a = l2_pool.tile([128, 512], FP32, tag="l2a", name="psum2a_t")[:]
                    psum2b = l2_pool.tile([128, 512], FP32, tag="l2b", name="psum2b_t")[:]
                    nc.tensor.matmul(psum2a[0:64], W2q[0:64], hk[0:64, 0:512], tile_position=(0, 0))
                    nc.tensor.matmul(psum2a[64:128], W2q[64:128], hk[64:128, 0:512], tile_position=(64, 64))
                    nc.tensor.matmul(psum2b[64:128], W2q[0:64], hk[0:64, 512:1024], tile_position=(0, 64))
                    nc.tensor.matmul(psum2b[0:64], W2q[64:128], hk[64:128, 512:1024], tile_position=(64, 0))
                    h2a = work_pool.tile([128, 512], BF16, tag="h2a", name="h2a_t")[:]
                    h2b = work_pool.tile([128, 512], BF16, tag="h2b", name="h2b_t")[:]
                    nc.scalar.activation(h2a, psum2a, AF.Relu, bias=bias2)
                    if sb % 2 == 0:
                        nc.scalar.activation(h2b, psum2b, AF.Relu, bias=bias2)
                    else:
                        nc.vector.tensor_scalar(h2b, psum2b, bias2, 0.0, add, max_)
                    h2_tiles[sb] = (h2a, h2b)

                def emit_l3(sb):
                    pair, k = divmod(sb, 2)
                    if k == 0:
                        psum3_tiles[pair] = l3_pool.tile([128, COLS], FP32, tag="l3", name="psum3_t")[:]
                    psum3 = psum3_tiles[pair]
                    h2a, h2b = h2_tiles.pop(sb)
                    ck = slice(512 * k, 512 * k + 512)
                    nc.tensor.matmul(psum3[0:32, ck], W3q[0:64], h2a[0:64], tile_position=(0, 0))
                    nc.tensor.matmul(psum3[32:64, ck], W3q[64:128], h2a[64:128], tile_position=(64, 32))
                    nc.tensor.matmul(psum3[64:96, ck], W3q[64:128], h2b[64:128], tile_position=(64, 64))
                    nc.tensor.matmul(psum3[96:128, ck], W3q[0:64], h2b[0:64], tile_position=(0, 96))

                def emit_sigmoid_dma(pair):
                    if pair % 2 == 0:
                        sig_tiles[pair // 2] = sig_pool.tile(
                            [128, 2 * COLS], BF16, tag="sig", bufs=3, name="sig_t"
                        )[:]
                    sig2 = sig_tiles[pair // 2]
                    psum3 = psum3_tiles.pop(pair)
                    half = COLS * (pair % 2)
                    nc.scalar.activation(
                        sig2[:, half : half + COLS], psum3, AF.Sigmoid, bias=bias3
                    )
                    if pair % 2 == 1:
                        for g in range(4):
                            nc.sync.dma_start(
                                out=d_out[pair // 2, g],
                                in_=sig2[32 * g : 32 * g + F],
                            )
                    # HAM keep-warm fillers into the PREVIOUS pair's psum3
                    # tile — its sigmoid finished a pair ago, so the WAR dep
                    # is already satisfied when the PE reaches these in the
                    # FIFO (no head-of-line stall). Garbage values are fine:
                    # the next user starts a fresh accumulation group.
                    filler_tiles[pair] = psum3
                    prev = filler_tiles.pop(pair - 1, None)
                    if prev is not None:
                        for _ in range(4):
                            nc.tensor.matmul(prev[:, 0:512], junk[:, 0:128], junk)

                h1_cur = None
                for sb in range(NSB):
                    pair, k = divmod(sb, 2)
                    if k == 0:
                        h1_cur = emit_s1(pair)
                    if sb >= 1:
                        emit_l3(sb - 1)
                    emit_l2_s2(sb, h1_cur[:, COLS * k : COLS * k + COLS])
                    if sb >= 1 and sb % 2 == 0:  # sb-1 was odd: its pair done
                        emit_sigmoid_dma((sb - 1) // 2)
                emit_l3(NSB - 1)
                emit_sigmoid_dma(NPAIR - 1)

    nc.compile()
    input_names = ["statesT", "Wpack", "biases"]
    return nc, input_names


def get_program():
    global _PROGRAM
    if _PROGRAM is None:
        _PROGRAM = _build_program()
    return _PROGRAM


def make_inputs(states, W1, b1, W2, b2, W3, b3):
    """Host-side prep: per-core statesT + shared packed weights/biases."""
    states = np.asarray(states, np.float32)
    W1 = np.asarray(W1, np.float32)
    W2 = np.asarray(W2, np.float32)
    W3 = np.asarray(W3, np.float32)
    b1 = np.asarray(b1, np.float32)
    b2 = np.asarray(b2, np.float32)
    b3 = np.asarray(b3, np.float32)

    Wpack = np.zeros((128, 224), NP_BF16)
    # Wl1 block: [48, 128] at cols 0:128
    Wpack[0:16, 0:64] = W1[:D].astype(NP_BF16)
    Wpack[0:16, 64:128] = W1[D:].astype(NP_BF16)
    Wpack[32:48, 0:64] = W1[:D].astype(NP_BF16)
    Wpack[32:48, 64:128] = W1[D:].astype(NP_BF16)
    # W2 block: [128, 64] at cols 128:192
    Wpack[0:64, 128:192] = W2.astype(NP_BF16)
    Wpack[64:128, 128:192] = W2.astype(NP_BF16)
    # W3 block: [128, 32] at cols 192:224 (cols 192:200 real)
    Wpack[0:64, 192:200] = W3.astype(NP_BF16)
    Wpack[64:128, 192:200] = W3.astype(NP_BF16)

    biases = np.zeros((128, 3), np.float32)
    biases[:, 0] = np.tile(b1, 2)
    biases[:, 1] = np.tile(b2, 2)
    biases[:, 2] = np.tile(np.concatenate([b3, np.zeros(24, np.float32)]), 4)

    shared = {"Wpack": Wpack, "biases": biases}

    in_maps = []
    for c in range(NCORES):
        # statesT[d, 32*l + i] = states[c, l, i, d]
        statesT = states[c].reshape(L * N, D).T.astype(NP_BF16)
        in_maps.append({"statesT": np.ascontiguousarray(statesT), **shared})
    return in_maps


def decode_output(raw):
    """Invert the device output layout -> [L, N, N, F] for one core.

    raw: [NPAIR//2, 4, F, 2, COLS] bf16; group g = 2*colhalf + block_parity,
    kp = pair%2, col = 512*k + q holds inner-sb k;
    l = 8*pg + 4*kp + 2*k + parity, pair-col = 512*colhalf + q = 32*i + j.
    """
    ov = raw.astype(np.float32)
    ov = ov.reshape(8, 2, 2, F, 2, 2, 512)           # [pg, h, par, f, kp, k, q]
    ov = ov.transpose(0, 4, 5, 2, 1, 6, 3)           # [pg, kp, k, par, h, q, f]
    return np.ascontiguousarray(ov.reshape(L, N, N, F))


def _ensure_ntff_hook():
    """Best-effort shim for the missing antenv.axon_hooks module so
    run_bass_kernel_spmd(trace=True) can capture NTFF profiles under axon."""
    import types

    try:
        from antenv.axon_hooks import get_axon_ntff_profile_hook  # noqa: F401
        return
    except ImportError:
        pass
    try:
        if "/root/.axon_site" not in sys.path:
            sys.path.insert(0, "/root/.axon_site")
        from trn_agent_boot.trn_boot import _ntff_profile_via_ctypes

        hook = _ntff_profile_via_ctypes("/opt/axon/libaxon_pjrt.so")
        import antenv

        mod = types.ModuleType("antenv.axon_hooks")
        mod._hook = hook
        mod.set_axon_ntff_profile_hook = lambda h: setattr(mod, "_hook", h)
        mod.get_axon_ntff_profile_hook = lambda: mod._hook
        sys.modules["antenv.axon_hooks"] = mod
        antenv.axon_hooks = mod
    except Exception as e:  # tracing is optional; never break the run
        print(f"ntff hook shim failed: {e}", file=sys.stderr)


def kernel(states, W1, b1, W2, b2, W3, b3):
    global LAST_RESULT
    nc, _ = get_program()
    if os.environ.get("KERNEL_TRACE"):
        _ensure_ntff_hook()
    in_maps = make_inputs(states, W1, b1, W2, b2, W3, b3)
    res = run_bass_kernel_spmd(
        nc,
        in_maps,
        core_ids=list(range(NCORES)),
        trace=bool(os.environ.get("KERNEL_TRACE")),
    )
    LAST_RESULT = res
    out = np.empty((B, L, N, N, F), np.float32)
    for c in range(NCORES):
        out[c] = decode_output(res.results[c]["out"])
    return out


# revision 35
# speedup vs baseline: 1.3273x; 1.3273x over previous
"""Trainium2 Bass kernel for pairwise-MLP GNN message passing.

Computation (per batch b, position l):
    x[i,j] = concat(states[l,i], states[l,j])           # [N,N,2D]
    out    = sigmoid(MLP(x))                            # [N,N,8], MLP: 32->64->64->8

Factorization used on device: the first linear layer splits into
A = states @ W1[:D] + b1 and B = states @ W1[D:], so
h1[i,j] = relu(A[i] + B[j]) — the N^2 expansion happens as a cheap
broadcast add on the vector engine instead of an N^2-row matmul.

Sharding: data-parallel over batch, core c <- batch c (8 cores, B=8).

Device layout (per core, L=64 l-blocks, 2 l-blocks = 1 "sb" superblock,
2 sbs = 1 "pair"):
  - features live on partitions: partitions 0:64 = even l-block of the sb,
    64:128 = odd l-block (via a device-side shifted DMA of states^T).
  - pair columns col = 32*i + j, 1024 per l-block.
  - L2/L3 use BLOCK-DIAGONAL weights ([[W,0],[0,W]]) so every matmul
    contracts the full 128 partitions — one full-width stream per 512
    columns instead of two half-width ones (the SBUF->PE bus moves 128
    partitions per cycle, so half-width matmuls waste half of it).
  - h2 eviction: column-half a always on ScalarE; column-half b
    alternates ScalarE (even sb) / VectorE (odd sb) for engine balance;
    each L3 chunk depends on exactly one evicting engine.
  - Output leaves the device as bf16 sigmoid(z) in [8 groups-of-2-pairs,
    4 f-groups, 8 f, 2, 1024] per core; the host upcasts and inverts the
    layout while unsharding.
"""

import os
import sys

import numpy as np

for _p in ("/opt/trn_rl_repo", "/root/.axon_site/_ro/trn_rl_repo"):
    if os.path.isdir(_p) and _p not in sys.path:
        sys.path.insert(0, _p)

from concourse import bacc, mybir, tile
from concourse.bass_utils import run_bass_kernel_spmd

B, L, N, D = 8, 64, 32, 16
H = 64            # hidden width (h1 and h2)
F = 8             # out_dim
NCORES = 8
NSB = L // 2      # 32 superblocks per core
NPAIR = NSB // 2  # 16 pairs per core
COLS = N * N      # 1024 pair columns per l-block

FP32 = mybir.dt.float32
BF16 = mybir.dt.bfloat16
NP_BF16 = mybir.dt.np(BF16)

_PROGRAM = None  # (nc, input_names)
LAST_RESULT = None  # BassKernelResults of the most recent kernel() call


def _build_program():
    nc = bacc.Bacc("TRN2", target_bir_lowering=False, debug=False)

    d_states = nc.dram_tensor("statesT", [16, 2048], BF16, kind="ExternalInput").ap()
    d_Wpack = nc.dram_tensor("Wpack", [128, 448], BF16, kind="ExternalInput").ap()
    d_biases = nc.dram_tensor("biases", [128, 3], FP32, kind="ExternalInput").ap()
    d_out = nc.dram_tensor(
        "out", [NPAIR // 2, 4, F, 2, COLS], BF16, kind="ExternalOutput"
    ).ap()

    add = mybir.AluOpType.add
    max_ = mybir.AluOpType.max
    AF = mybir.ActivationFunctionType

    with tile.TileContext(nc) as tc:
        with tc.tile_pool(name="const", bufs=1) as const_pool:
            statesQ = const_pool.tile([32, 2048], BF16, name="statesQ_t")[:]
            Wpack = const_pool.tile([128, 448], BF16, name="Wpack_t")[:]
            biases = const_pool.tile([128, 3], FP32, name="biases_t")[:]
            WA = Wpack[0:32, 0:128]     # block-diag W1a (A halves, 2 blocks)
            WB = Wpack[0:32, 128:256]   # block-diag W1b
            W2bd = Wpack[:, 256:384]    # block-diag W2 [128, 128]
            W3bd = Wpack[:, 384:448]    # block-diag W3 [128, 64]
            bias1 = biases[:, 0:1]
            bias2 = biases[:, 1:2]
            bias3 = biases[:, 2:3]
            A2dup = const_pool.tile([128, 2 * COLS], BF16, name="A2dup_t")[:]
            B2s = const_pool.tile([128, COLS], BF16, name="B2s_t")[:]
            junk = const_pool.tile([128, 512], BF16, name="junk_t")[:]

            # statesQ rows 0:16 = statesT, rows 16:32 = statesT shifted left
            # by one l-slot (32 cols) — built from the same DRAM region,
            # split so the first L1 matmuls start before the full load.
            nc.sync.dma_start(out=statesQ[0:16, 0:1024], in_=d_states[:, 0:1024])
            nc.sync.dma_start(out=statesQ[16:32, 0:1024], in_=d_states[:, 32:1056])
            nc.sync.dma_start(out=Wpack, in_=d_Wpack)
            nc.sync.dma_start(out=biases, in_=d_biases)
            nc.sync.dma_start(out=statesQ[0:16, 1024:2048], in_=d_states[:, 1024:2048])
            nc.sync.dma_start(out=statesQ[16:32, 1024:2016], in_=d_states[:, 1056:2048])

            # ---- Layer 1: A2/B2 = per-agent halves of the first linear layer.
            # Block-diag lhsT gives [128 parts: 0:64 even l-block, 64:128 odd]
            # in a single matmul per (A/B, statesQ-half).
            with tc.tile_pool(name="abps", bufs=1, space="PSUM") as ab_pool:
                A2ps = ab_pool.tile([128, COLS], FP32, tag="a2", name="A2ps_t")[:]
                B2ps = ab_pool.tile([128, COLS], FP32, tag="b2", name="B2ps_t")[:]
                warm = ab_pool.tile([128, 512], FP32, tag="warm", name="warm_t")[:]
                rhs = statesQ.rearrange("p (s c) -> p s c", s=32)
                # HAM warmup: ~4.7us of back-to-back FULL-ARRAY matmuls
                # (junk data, no DMA dependency) flips the PE clock gate to
                # 8/8 during the input-DMA window. Results are never read.
                nc.gpsimd.memset(junk, 0.5)
                for _ in range(11):
                    nc.tensor.matmul(warm, junk[:, 0:128], junk)
                for sbh in (0, 1):
                    for lhsT, ps in ((WA, A2ps), (WB, B2ps)):
                        nc.tensor.matmul(
                            ps[:, 512 * sbh : 512 * sbh + 512],
                            lhsT,
                            rhs[:, 16 * sbh : 16 * sbh + 16, 0:32],
                        )
                # Bridge warmups: keep the PE busy through the pipeline-fill
                # window so the HAM never sees an idle window and re-throttles.
                for _ in range(14):
                    nc.tensor.matmul(warm, junk[:, 0:128], junk)
                # Evict A2 twice (duplicated pairs so the later broadcast add
                # keeps an innermost unit stride), folding in b1. First 256
                # cols (pairs 0-3) go as small early ops (ScalarE/VectorE in
                # parallel); remainders run on ScalarE, keeping VectorE free
                # for the first expansion adds.
                dupview = A2dup.rearrange("p (c two) -> p two c", two=2)
                nc.scalar.activation(dupview[:, 0, 0:256], A2ps[:, 0:256], AF.Identity, bias=bias1)
                nc.vector.tensor_scalar(dupview[:, 1, 0:256], A2ps[:, 0:256], bias1, 0.0, add, add)
                nc.vector.tensor_copy(B2s[:, 0:256], B2ps[:, 0:256])
                nc.scalar.activation(dupview[:, 0, 256:1024], A2ps[:, 256:1024], AF.Identity, bias=bias1)
                nc.scalar.activation(dupview[:, 1, 256:1024], A2ps[:, 256:1024], AF.Identity, bias=bias1)
                nc.scalar.activation(B2s[:, 256:1024], B2ps[:, 256:1024], AF.Identity)

            with (
                tc.tile_pool(name="work", bufs=6) as work_pool,
                tc.tile_pool(name="sigp", bufs=2) as sig_pool,
                tc.tile_pool(name="l2ps", bufs=2, space="PSUM") as l2_pool,
                tc.tile_pool(name="l3ps", bufs=2, space="PSUM") as l3_pool,
            ):
                # Software pipeline, 1 sb deep: PE order is
                # L2(0), L3(0), L2(1), L3(1), ... so L3(sb) (which needs
                # S2(sb)'s eviction) never blocks L2(sb+1) in the strict-FIFO
                # PE queue.
                h2_tiles = {}     # sb -> h2 AP
                psum3_tiles = {}  # pair -> psum3 AP
                sig_tiles = {}    # pairgroup -> sig AP

                def emit_s1(pair):
                    h1pre = work_pool.tile([128, 2 * COLS], BF16, tag="h1pre", name="h1pre_t")[:]
                    h1 = work_pool.tile([128, 2 * COLS], BF16, tag="h1", name="h1_t")[:]
                    a_in = (
                        A2dup[:, 128 * pair : 128 * pair + 128]
                        .rearrange("p (s i two) -> p s i two", s=2, two=2)
                        .unsqueeze(3)
                        .broadcast_to([128, 2, 32, 16, 2])
                    )
                    b_in = (
                        B2s[:, 64 * pair : 64 * pair + 64]
                        .rearrange("p (s jh jl) -> p s jh jl", s=2, jl=2)
                        .unsqueeze(2)
                        .broadcast_to([128, 2, 32, 16, 2])
                    )
                    h1pre_v = h1pre.rearrange(
                        "p (s i jh jl) -> p s i jh jl", s=2, i=32, jl=2
                    )
                    nc.vector.tensor_add(h1pre_v, a_in, b_in)
                    nc.vector.tensor_scalar_max(h1, h1pre, 0.0)
                    return h1

                def emit_l2_s2(sb, hk):
                    # Two full-width (k=128 block-diag) matmuls per sb.
                    # Column-half a: evicted by ScalarE; column-half b
                    # alternates ScalarE (even sb) / VectorE (odd sb).
                    psum2a = l2_pool.tile([128, 512], FP32, tag="l2a", name="psum2a_t")[:]
                    psum2b = l2_pool.tile([128, 512], FP32, tag="l2b", name="psum2b_t")[:]
                    nc.tensor.matmul(psum2a, W2bd, hk[:, 0:512])
                    nc.tensor.matmul(psum2b, W2bd, hk[:, 512:1024])
                    h2a = work_pool.tile([128, 512], BF16, tag="h2a", name="h2a_t")[:]
                    h2b = work_pool.tile([128, 512], BF16, tag="h2b", name="h2b_t")[:]
                    nc.scalar.activation(h2a, psum2a, AF.Relu, bias=bias2)
                    if sb % 2 == 0:
                        nc.scalar.activation(h2b, psum2b, AF.Relu, bias=bias2)
                    else:
                        nc.vector.tensor_scalar(h2b, psum2b, bias2, 0.0, add, max_)
                    h2_tiles[sb] = (h2a, h2b)

                def emit_l3(sb):
                    pair, k = divmod(sb, 2)
                    if k == 0:
                        psum3_tiles[pair] = l3_pool.tile([128, COLS], FP32, tag="l3", name="psum3_t")[:]
                    psum3 = psum3_tiles[pair]
                    h2a, h2b = h2_tiles.pop(sb)
                    ck = slice(512 * k, 512 * k + 512)
                    nc.tensor.matmul(psum3[0:64, ck], W3bd, h2a)
                    nc.tensor.matmul(psum3[64:128, ck], W3bd, h2b)

                def emit_sigmoid_dma(pair):
                    if pair % 2 == 0:
                        sig_tiles[pair // 2] = sig_pool.tile(
                            [128, 2 * COLS], BF16, tag="sig", bufs=3, name="sig_t"
                        )[:]
                    sig2 = sig_tiles[pair // 2]
                    psum3 = psum3_tiles.pop(pair)
                    half = COLS * (pair % 2)
                    nc.scalar.activation(
                        sig2[:, half : half + COLS], psum3, AF.Sigmoid, bias=bias3
                    )
                    if pair % 2 == 1:
                        for g in range(4):
                            nc.sync.dma_start(
                                out=d_out[pair // 2, g],
                                in_=sig2[32 * g : 32 * g + F],
                            )

                h1_cur = None
                for sb in range(NSB):
                    pair, k = divmod(sb, 2)
                    if k == 0:
                        h1_cur = emit_s1(pair)
                    if sb >= 1:
                        emit_l3(sb - 1)
                    emit_l2_s2(sb, h1_cur[:, COLS * k : COLS * k + COLS])
                    if sb >= 1 and sb % 2 == 0:  # sb-1 was odd: its pair done
                        emit_sigmoid_dma((sb - 1) // 2)
                emit_l3(NSB - 1)
                emit_sigmoid_dma(NPAIR - 1)

    nc.compile()
    input_names = ["statesT", "Wpack", "biases"]
    return nc, input_names


def get_program():
    global _PROGRAM
    if _PROGRAM is None:
        _PROGRAM = _build_program()
    return _PROGRAM


def make_inputs(states, W1, b1, W2, b2, W3, b3):
    """Host-side prep: per-core statesT + shared packed weights/biases."""
    states = np.asarray(states, np.float32)
    W1 = np.asarray(W1, np.float32)
    W2 = np.asarray(W2, np.float32)
    W3 = np.asarray(W3, np.float32)
    b1 = np.asarray(b1, np.float32)
    b2 = np.asarray(b2, np.float32)
    b3 = np.asarray(b3, np.float32)

    W1a = W1[:D].astype(NP_BF16)   # [16, 64]
    W1b = W1[D:].astype(NP_BF16)

    Wpack = np.zeros((128, 448), NP_BF16)
    # block-diag W1a at cols 0:128: even l-block -> out 0:64, odd -> 64:128
    Wpack[0:16, 0:64] = W1a
    Wpack[16:32, 64:128] = W1a
    # block-diag W1b at cols 128:256
    Wpack[0:16, 128:192] = W1b
    Wpack[16:32, 192:256] = W1b
    # block-diag W2 at cols 256:384
    Wpack[0:64, 256:320] = W2.astype(NP_BF16)
    Wpack[64:128, 320:384] = W2.astype(NP_BF16)
    # block-diag W3 at cols 384:448 (8 of each 32 out cols real)
    Wpack[0:64, 384:392] = W3.astype(NP_BF16)
    Wpack[64:128, 416:424] = W3.astype(NP_BF16)

    biases = np.zeros((128, 3), np.float32)
    biases[:, 0] = np.tile(b1, 2)
    biases[:, 1] = np.tile(b2, 2)
    biases[:, 2] = np.tile(np.concatenate([b3, np.zeros(24, np.float32)]), 4)

    shared = {"Wpack": Wpack, "biases": biases}

    in_maps = []
    for c in range(NCORES):
        # statesT[d, 32*l + i] = states[c, l, i, d]
        statesT = states[c].reshape(L * N, D).T.astype(NP_BF16)
        in_maps.append({"statesT": np.ascontiguousarray(statesT), **shared})
    return in_maps


def decode_output(raw):
    """Invert the device output layout -> [L, N, N, F] for one core.

    raw: [NPAIR//2, 4, F, 2, COLS] bf16; group g = 2*colhalf + block_parity,
    kp = pair%2, col = 512*k + q holds inner-sb k;
    l = 8*pg + 4*kp + 2*k + parity, pair-col = 512*colhalf + q = 32*i + j.
    """
    ov = raw.astype(np.float32)
    ov = ov.reshape(8, 2, 2, F, 2, 2, 512)           # [pg, h, par, f, kp, k, q]
    ov = ov.transpose(0, 4, 5, 2, 1, 6, 3)           # [pg, kp, k, par, h, q, f]
    return np.ascontiguousarray(ov.reshape(L, N, N, F))


def _ensure_ntff_hook():
    """Best-effort shim for the missing antenv.axon_hooks module so
    run_bass_kernel_spmd(trace=True) can capture NTFF profiles under axon."""
    import types

    try:
        from antenv.axon_hooks import get_axon_ntff_profile_hook  # noqa: F401
        return
    except ImportError:
        pass
    try:
        if "/root/.axon_site" not in sys.path:
            sys.path.insert(0, "/root/.axon_site")
        from trn_agent_boot.trn_boot import _ntff_profile_via_ctypes

        hook = _ntff_profile_via_ctypes("/opt/axon/libaxon_pjrt.so")
        import antenv

        mod = types.ModuleType("antenv.axon_hooks")
        mod._hook = hook
        mod.set_axon_ntff_profile_hook = lambda h: setattr(mod, "_hook", h)
        mod.get_axon_ntff_profile_hook = lambda: mod._hook
        sys.modules["antenv.axon_hooks"] = mod
        antenv.axon_hooks = mod
    except Exception as e:  # tracing is optional; never break the run
        print(f"ntff hook shim failed: {e}", file=sys.stderr)


def kernel(states, W1, b1, W2, b2, W3, b3):
    global LAST_RESULT
    nc, _ = get_program()
    if os.environ.get("KERNEL_TRACE"):
        _ensure_ntff_hook()
    in_maps = make_inputs(states, W1, b1, W2, b2, W3, b3)
    res = run_bass_kernel_spmd(
        nc,
        in_maps,
        core_ids=list(range(NCORES)),
        trace=bool(os.environ.get("KERNEL_TRACE")),
    )
    LAST_RESULT = res
    out = np.empty((B, L, N, N, F), np.float32)
    for c in range(NCORES):
        out[c] = decode_output(res.results[c]["out"])
    return out
